# revision 18
# baseline (speedup 1.0000x reference)
"""Multi-head attention (B=4, S=2048, D=1024, H=16) on 8 trn2 NeuronCores.

Sharding: core c handles batch b = c//2 and head-group g = c%2 (8 heads,
512 of the 1024 embedding dims). W_Q/W_K/W_V are column-sharded, W_O is
row-sharded; each core produces a partial [D, S] (transposed) output and the
host sums the two partials per batch (the "all-reduce" of the TP split).

Per-core layout choices:
  - Host passes X^T ([D, S]) so the contraction dim of every projection
    matmul is already on partitions.
  - q/k are produced transposed ([e, s]); v is produced natural ([s, e])
    with a ones column appended per head (65-wide head stride).
  - scores are computed transposed: scoresT[k, q] = k_blk @ q^T, so the
    exp'd scores tile is directly the stationary operand of the AV matmul
    (no on-chip transpose anywhere).
  - Masking: reference sets masked scores to 1e-9 pre-softmax. We multiply
    scoresT by a per-partition {0,1} mask inside the Exp activation
    (exp(0) = 1.0 = exp(1e-9) in fp32), which is exact.
  - Softmax denominator: the ones column of v gives row 64 of the AV psum
    accumulator = sum_k exp(scoresT[k, q]).
  - No max-subtraction: scores ~ N(0,1), exp never overflows fp32.
"""

import numpy as np
import ml_dtypes

import concourse.bass as bass
import concourse.tile as tile
from concourse import mybir
from concourse.bass_utils import run_bass_kernel_spmd
from concourse.vector_clock import ScopedClock

# The walrus build in this container rejects instructions carrying more than
# one semaphore wait ("Too many sync wait commands"), while Tile's scheduler
# freely attaches several. Post-pass: hoist extra waits onto nop instructions
# injected just before the offender on the same engine queue (engines execute
# their queue in order, so the semantics are identical).
def _split_multi_waits(nc, limit=1):
    fn = nc.m.functions[0]
    for b in fn.blocks:
        new = []
        changed = False
        for inst in b.instructions:
            si = inst.sync_info
            waits = list(si.on_wait) if si is not None else []
            if len(waits) > limit:
                for w in waits[:-limit]:
                    nop = mybir.InstNoOp(
                        name=nc.get_next_instruction_name(), ins=[], outs=[]
                    )
                    nop.engine = inst.engine
                    nop.sync_info = mybir.SyncInfo(on_wait=[w], on_update=[])
                    nc.register_instruction(nop)
                    new.append(nop)
                inst.sync_info = mybir.SyncInfo(
                    on_wait=waits[-limit:], on_update=si.on_update
                )
                changed = True
            new.append(inst)
        if changed:
            b.instructions = new

B, S, D, H = 4, 2048, 1024, 16
DH = D // H            # 64 head dim
HL = H // 2            # 8 heads per core
E = HL * DH            # 512 per-core head width
P = 128
SC = 512               # psum bank width in f32 (max matmul N)
NCH = S // SC          # 4 projection chunks
SCE = 1024             # attention q-chunk (ACT overhead amortization)
NCHE = S // SCE        # 2 attention chunks
NSUB = SCE // SC       # 2 matmul sub-chunks per attention chunk
KB = S // P            # 16 k-blocks
DT = D // P            # 8 contraction tiles
ET = E // P            # 4 e-tiles
OB = D // P            # 8 output-row blocks
VW = DH + 1            # 65: head width in v (with ones column)

BF16 = mybir.dt.bfloat16
F32 = mybir.dt.float32
npbf16 = ml_dtypes.bfloat16


def build_nc():
    nc = bass.Bass()
    xqT_d = nc.dram_tensor("xqT", [D, S], BF16, kind="ExternalInput")
    xkT_d = nc.dram_tensor("xkT", [D, S], BF16, kind="ExternalInput")
    xvT_d = nc.dram_tensor("xvT", [D, S], BF16, kind="ExternalInput")
    wq_d = nc.dram_tensor("wq", [D, E], BF16, kind="ExternalInput")
    wk_d = nc.dram_tensor("wk", [D, E], BF16, kind="ExternalInput")
    wv_d = nc.dram_tensor("wv", [D, E], BF16, kind="ExternalInput")
    wo_d = nc.dram_tensor("wo", [E, D], BF16, kind="ExternalInput")
    sel_d = nc.dram_tensor("sel", [P, KB], F32, kind="ExternalInput")
    outT_d = nc.dram_tensor("outT", [D, S], F32, kind="ExternalOutput")

    with tile.TileContext(nc) as tc:
        with (
            tc.tile_pool(name="wpool", bufs=1) as wpool,
            tc.tile_pool(name="xpool", bufs=12) as xpool,
            tc.tile_pool(name="qkv", bufs=1) as qkv,
            tc.tile_pool(name="expp", bufs=6) as expp,
            tc.tile_pool(name="ctxp", bufs=8) as ctxp,
            tc.tile_pool(name="recp", bufs=2) as recp,
            tc.tile_pool(name="bcp", bufs=2) as bcp,
            tc.tile_pool(name="outp", bufs=4) as outp,
            tc.tile_pool(name="ps_sc", bufs=2, space="PSUM") as ps_sc,
            tc.tile_pool(name="ps_ctx", bufs=2, space="PSUM") as ps_ctx,
        ):
            wq_sb = wpool.tile([P, DT, E], BF16)
            wk_sb = wpool.tile([P, DT, E], BF16)
            wv_sb = wpool.tile([P, DT, E], BF16)
            wo_sb = wpool.tile([P, ET, D], BF16)
            sel_sb = wpool.tile([P, KB], F32)
            nc.sync.dma_start(wq_sb, wq_d.rearrange("(t p) n -> p t n", p=P))
            nc.sync.dma_start(wk_sb, wk_d.rearrange("(t p) n -> p t n", p=P))
            nc.sync.dma_start(wv_sb, wv_d.rearrange("(t p) n -> p t n", p=P))
            nc.sync.dma_start(wo_sb, wo_d.rearrange("(t p) n -> p t n", p=P))
            nc.sync.dma_start(sel_sb, sel_d[:, :])

            qT_sb = qkv.tile([P, ET, S], BF16)
            kT_sb = qkv.tile([P, ET, S], BF16)
            v_sb = qkv.tile([P, KB, HL * VW], BF16)
            # ones columns of v (one per head)
            nc.vector.memset(
                v_sb.rearrange("p t (h c) -> p t h c", c=VW)[:, :, :, DH : DH + 1], 1.0
            )

            def proj_T(x_dram, w_sb, out_sb):
                # out_sb[e, s] = (X @ W).T ; lhsT = W[d, e], rhs = X.T[d, s]
                xt = []
                for dt in range(DT):
                    xtile = xpool.tile([P, S], BF16, tag="xt")
                    nc.sync.dma_start(xtile, x_dram[dt * P : (dt + 1) * P, :])
                    xt.append(xtile)
                for et in range(ET):
                    for sc in range(NCH):
                        ps = ps_sc.tile([P, SC], F32, tag="sc")
                        for dt in range(DT):
                            nc.tensor.matmul(
                                ps,
                                lhsT=w_sb[:, dt, et * P : (et + 1) * P],
                                rhs=xt[dt][:, sc * SC : (sc + 1) * SC],
                                start=(dt == 0),
                                stop=(dt == DT - 1),
                            )
                        nc.vector.tensor_copy(
                            out_sb[:, et, sc * SC : (sc + 1) * SC], ps
                        )

            proj_T(xqT_d, wq_sb, qT_sb)
            proj_T(xkT_d, wk_sb, kT_sb)

            # v natural: v[s, e] ; lhsT = X.T[d, s-block], rhs = W[d, e]
            xt = []
            for dt in range(DT):
                xtile = xpool.tile([P, S], BF16, tag="xt")
                nc.sync.dma_start(xtile, xvT_d[dt * P : (dt + 1) * P, :])
                xt.append(xtile)
            for st in range(KB):
                ps = ps_sc.tile([P, SC], F32, tag="sc")
                for dt in range(DT):
                    nc.tensor.matmul(
                        ps,
                        lhsT=xt[dt][:, st * P : (st + 1) * P],
                        rhs=wv_sb[:, dt, :],
                        start=(dt == 0),
                        stop=(dt == DT - 1),
                    )
                nc.vector.tensor_copy(
                    v_sb[:, st].rearrange("p (h c) -> p h c", c=VW)[:, :, 0:DH],
                    ps.rearrange("p (h c) -> p h c", c=DH),
                )

            # attention + output projection, per q-chunk of SCE
            for ch in range(NCHE):
                qs = slice(ch * SCE, (ch + 1) * SCE)
                ctx_tiles = []
                for h in range(HL):
                    et, ro = h // 2, (h % 2) * DH
                    ctx_ps = ps_ctx.tile([VW, SCE], F32, tag="ctx")
                    for kb in range(KB):
                        sc_ps = ps_sc.tile([P, SCE], F32, tag="sc")
                        for j in range(NSUB):
                            nc.tensor.matmul(
                                sc_ps[:, j * SC : (j + 1) * SC],
                                lhsT=kT_sb[ro : ro + DH, et, kb * P : (kb + 1) * P],
                                rhs=qT_sb[
                                    ro : ro + DH,
                                    et,
                                    ch * SCE + j * SC : ch * SCE + (j + 1) * SC,
                                ],
                                start=True,
                                stop=True,
                            )
                        ex = expp.tile([P, SCE], BF16, tag="ex")
                        nc.scalar.activation(
                            ex,
                            sc_ps,
                            mybir.ActivationFunctionType.Exp,
                            scale=sel_sb[:, kb : kb + 1],
                        )
                        for j in range(NSUB):
                            nc.tensor.matmul(
                                ctx_ps[:, j * SC : (j + 1) * SC],
                                lhsT=v_sb[:, kb, h * VW : (h + 1) * VW],
                                rhs=ex[:, j * SC : (j + 1) * SC],
                                start=(kb == 0),
                                stop=(kb == KB - 1),
                            )
                    rec = recp.tile([1, SCE], F32, tag="rec")
                    nc.vector.reciprocal(rec, ctx_ps[DH : DH + 1, :])
                    bc_sb = bcp.tile([DH, SCE], F32, tag="bc")
                    rec_bcast = bass.AP(
                        tensor=rec.tensor,
                        offset=rec.offset,
                        ap=[list(rec.ap[0]), [0, DH], list(rec.ap[1])],
                    )
                    nc.scalar.dma_start(bc_sb, rec_bcast)
                    if h % 2 == 0:
                        ct = ctxp.tile([P, SCE], BF16, tag="ct")
                        ctx_tiles.append(ct)
                    nc.vector.tensor_mul(
                        ctx_tiles[et][ro : ro + DH, :], ctx_ps[0:DH, :], bc_sb
                    )
                for ob in range(OB):
                    for j in range(NSUB):
                        ps = ps_sc.tile([P, SC], F32, tag="sc")
                        for et in range(ET):
                            nc.tensor.matmul(
                                ps,
                                lhsT=wo_sb[:, et, ob * P : (ob + 1) * P],
                                rhs=ctx_tiles[et][:, j * SC : (j + 1) * SC],
                                start=(et == 0),
                                stop=(et == ET - 1),
                            )
                        ost = outp.tile([P, SC], F32, tag="ost")
                        nc.vector.tensor_copy(ost, ps)
                        nc.gpsimd.dma_start(
                            outT_d[
                                ob * P : (ob + 1) * P,
                                ch * SCE + j * SC : ch * SCE + (j + 1) * SC,
                            ],
                            ost,
                        )

    _split_multi_waits(nc)
    return nc


def make_in_maps(Q, K, V, valid_lens, Wq, Wk, Wv, Wo):
    """Host-side sharding. Returns 8 in_maps (core c -> batch c//2, group c%2)."""
    xT = {}
    sel = {}
    for b in range(B):
        xT[b] = (
            np.ascontiguousarray(Q[b].T).astype(npbf16),
            np.ascontiguousarray(K[b].T).astype(npbf16),
            np.ascontiguousarray(V[b].T).astype(npbf16),
        )
        s = (np.arange(S) < valid_lens[b]).astype(np.float32)
        sel[b] = np.ascontiguousarray(s.reshape(KB, P).T)
    wshard = {}
    for g in range(2):
        cols = slice(g * E, (g + 1) * E)
        wshard[g] = (
            (Wq[:, cols] / 8.0).astype(npbf16),
            Wk[:, cols].astype(npbf16),
            Wv[:, cols].astype(npbf16),
            np.ascontiguousarray(Wo[cols, :]).astype(npbf16),
        )
    in_maps = []
    for c in range(8):
        b, g = c // 2, c % 2
        wq, wk, wv, wo = wshard[g]
        in_maps.append(
            {
                "xqT": xT[b][0],
                "xkT": xT[b][1],
                "xvT": xT[b][2],
                "wq": wq,
                "wk": wk,
                "wv": wv,
                "wo": wo,
                "sel": sel[b],
            }
        )
    return in_maps


_NC_CACHE = []


def kernel(Q, K, V, valid_lens, Wq, Wk, Wv, Wo):
    Q = np.asarray(Q, dtype=np.float32)
    K = np.asarray(K, dtype=np.float32)
    V = np.asarray(V, dtype=np.float32)
    Wq = np.asarray(Wq, dtype=np.float32)
    Wk = np.asarray(Wk, dtype=np.float32)
    Wv = np.asarray(Wv, dtype=np.float32)
    Wo = np.asarray(Wo, dtype=np.float32)
    valid_lens = np.asarray(valid_lens)

    in_maps = make_in_maps(Q, K, V, valid_lens, Wq, Wk, Wv, Wo)
    if not _NC_CACHE:
        _NC_CACHE.append(build_nc())
    nc = _NC_CACHE[0]
    res = run_bass_kernel_spmd(nc, in_maps, core_ids=list(range(8)))
    outs = [r["outT"] for r in res.results]
    out = np.empty((B, S, D), np.float32)
    for b in range(B):
        out[b] = (outs[2 * b] + outs[2 * b + 1]).T
    return out


# revision 19
# speedup vs baseline: 1.2018x; 1.2018x over previous
"""Multi-head attention (B=4, S=2048, D=1024, H=16) on 8 trn2 NeuronCores.

Sharding: core c handles batch b = c//2 and head-group g = c%2 (8 heads,
512 of the 1024 embedding dims). W_Q/W_K/W_V are column-sharded, W_O is
row-sharded; each core produces a partial [D, S] (transposed) output and the
host sums the two partials per batch (the "all-reduce" of the TP split).

Per-core layout choices:
  - Host passes X^T ([D, S]) so the contraction dim of every projection
    matmul is already on partitions.
  - q/k are produced transposed ([e, s]); v is produced natural ([s, e])
    with a ones column appended per head (65-wide head stride).
  - scores are computed transposed: scoresT[k, q] = k_blk @ q^T, so the
    exp'd scores tile is directly the stationary operand of the AV matmul
    (no on-chip transpose anywhere).
  - Masking: reference sets masked scores to 1e-9 pre-softmax. We multiply
    scoresT by a per-partition {0,1} mask inside the Exp activation
    (exp(0) = 1.0 = exp(1e-9) in fp32), which is exact.
  - Softmax denominator: the ones column of v gives row 64 of the AV psum
    accumulator = sum_k exp(scoresT[k, q]).
  - No max-subtraction: scores ~ N(0,1), exp never overflows fp32.
"""

import numpy as np
import ml_dtypes

import concourse.bass as bass
import concourse.tile as tile
from concourse import mybir
from concourse.bass_utils import run_bass_kernel_spmd
from concourse.vector_clock import ScopedClock

# The walrus build in this container rejects instructions carrying more than
# one semaphore wait ("Too many sync wait commands"), while Tile's scheduler
# freely attaches several. Post-pass: hoist extra waits onto nop instructions
# injected just before the offender on the same engine queue (engines execute
# their queue in order, so the semantics are identical).
def _split_multi_waits(nc, limit=1):
    fn = nc.m.functions[0]
    for b in fn.blocks:
        new = []
        changed = False
        for inst in b.instructions:
            si = inst.sync_info
            waits = list(si.on_wait) if si is not None else []
            if len(waits) > limit:
                for w in waits[:-limit]:
                    nop = mybir.InstNoOp(
                        name=nc.get_next_instruction_name(), ins=[], outs=[]
                    )
                    nop.engine = inst.engine
                    nop.sync_info = mybir.SyncInfo(on_wait=[w], on_update=[])
                    nc.register_instruction(nop)
                    new.append(nop)
                inst.sync_info = mybir.SyncInfo(
                    on_wait=waits[-limit:], on_update=si.on_update
                )
                changed = True
            new.append(inst)
        if changed:
            b.instructions = new

B, S, D, H = 4, 2048, 1024, 16
DH = D // H            # 64 head dim
HL = H // 2            # 8 heads per core
E = HL * DH            # 512 per-core head width
P = 128
SC = 512               # psum bank width in f32 (max matmul N)
NCH = S // SC          # 4 projection chunks
SCE = 1024             # attention q-chunk (ACT overhead amortization)
NCHE = S // SCE        # 2 attention chunks
NSUB = SCE // SC       # 2 matmul sub-chunks per attention chunk
KB = S // P            # 16 k-blocks
DT = D // P            # 8 contraction tiles
ET = E // P            # 4 e-tiles
OB = D // P            # 8 output-row blocks
VW = DH + 1            # 65: head width in v (with ones column)

BF16 = mybir.dt.bfloat16
F32 = mybir.dt.float32
npbf16 = ml_dtypes.bfloat16


def build_nc():
    nc = bass.Bass()
    xqT_d = nc.dram_tensor("xqT", [D, S], BF16, kind="ExternalInput")
    xkT_d = nc.dram_tensor("xkT", [D, S], BF16, kind="ExternalInput")
    xvT_d = nc.dram_tensor("xvT", [D, S], BF16, kind="ExternalInput")
    wq_d = nc.dram_tensor("wq", [D, E], BF16, kind="ExternalInput")
    wk_d = nc.dram_tensor("wk", [D, E], BF16, kind="ExternalInput")
    wv_d = nc.dram_tensor("wv", [D, E], BF16, kind="ExternalInput")
    wo_d = nc.dram_tensor("wo", [E, D], BF16, kind="ExternalInput")
    sel_d = nc.dram_tensor("sel", [P, KB], F32, kind="ExternalInput")
    outT_d = nc.dram_tensor("outT", [D, S], F32, kind="ExternalOutput")

    with tile.TileContext(nc) as tc:
        with (
            tc.tile_pool(name="wpool", bufs=1) as wpool,
            tc.tile_pool(name="xpool", bufs=12) as xpool,
            tc.tile_pool(name="qkv", bufs=1) as qkv,
            tc.tile_pool(name="expp", bufs=6) as expp,
            tc.tile_pool(name="ctxp", bufs=8) as ctxp,
            tc.tile_pool(name="recp", bufs=2) as recp,
            tc.tile_pool(name="bcp", bufs=2) as bcp,
            tc.tile_pool(name="outp", bufs=4) as outp,
            tc.tile_pool(name="ps_sc", bufs=2, space="PSUM") as ps_sc,
            tc.tile_pool(name="ps_ctx", bufs=2, space="PSUM") as ps_ctx,
        ):
            wq_sb = wpool.tile([P, DT, E], BF16)
            wk_sb = wpool.tile([P, DT, E], BF16)
            wv_sb = wpool.tile([P, DT, E], BF16)
            wo_sb = wpool.tile([P, ET, D], BF16)
            sel_sb = wpool.tile([P, KB], F32)
            nc.sync.dma_start(wq_sb, wq_d.rearrange("(t p) n -> p t n", p=P))
            nc.sync.dma_start(wk_sb, wk_d.rearrange("(t p) n -> p t n", p=P))
            nc.sync.dma_start(wv_sb, wv_d.rearrange("(t p) n -> p t n", p=P))
            nc.sync.dma_start(wo_sb, wo_d.rearrange("(t p) n -> p t n", p=P))
            nc.sync.dma_start(sel_sb, sel_d[:, :])

            qT_sb = qkv.tile([P, ET, S], BF16)
            kT_sb = qkv.tile([P, ET, S], BF16)
            v_sb = qkv.tile([P, KB, HL * VW], BF16)
            # ones columns of v (one per head)
            nc.vector.memset(
                v_sb.rearrange("p t (h c) -> p t h c", c=VW)[:, :, :, DH : DH + 1], 1.0
            )

            def proj_T(x_dram, w_sb, out_sb):
                # out_sb[e, s] = (X @ W).T ; lhsT = W[d, e], rhs = X.T[d, s]
                xt = []
                for dt in range(DT):
                    xtile = xpool.tile([P, S], BF16, tag="xt")
                    nc.sync.dma_start(xtile, x_dram[dt * P : (dt + 1) * P, :])
                    xt.append(xtile)
                for et in range(ET):
                    for sc in range(NCH):
                        ps = ps_sc.tile([P, SC], F32, tag="sc")
                        for dt in range(DT):
                            nc.tensor.matmul(
                                ps,
                                lhsT=w_sb[:, dt, et * P : (et + 1) * P],
                                rhs=xt[dt][:, sc * SC : (sc + 1) * SC],
                                start=(dt == 0),
                                stop=(dt == DT - 1),
                            )
                        nc.vector.tensor_copy(
                            out_sb[:, et, sc * SC : (sc + 1) * SC], ps
                        )

            proj_T(xqT_d, wq_sb, qT_sb)
            proj_T(xkT_d, wk_sb, kT_sb)

            # v natural: v[s, e] ; lhsT = X.T[d, s-block], rhs = W[d, e]
            xt = []
            for dt in range(DT):
                xtile = xpool.tile([P, S], BF16, tag="xt")
                nc.sync.dma_start(xtile, xvT_d[dt * P : (dt + 1) * P, :])
                xt.append(xtile)
            for st in range(KB):
                ps = ps_sc.tile([P, SC], F32, tag="sc")
                for dt in range(DT):
                    nc.tensor.matmul(
                        ps,
                        lhsT=xt[dt][:, st * P : (st + 1) * P],
                        rhs=wv_sb[:, dt, :],
                        start=(dt == 0),
                        stop=(dt == DT - 1),
                    )
                nc.vector.tensor_copy(
                    v_sb[:, st].rearrange("p (h c) -> p h c", c=VW)[:, :, 0:DH],
                    ps.rearrange("p (h c) -> p h c", c=DH),
                )

            # attention + output projection, per q-chunk of SCE
            for ch in range(NCHE):
                qs = slice(ch * SCE, (ch + 1) * SCE)
                ctx_tiles = []
                for h in range(HL):
                    et, ro = h // 2, (h % 2) * DH
                    ctx_ps = ps_ctx.tile([VW, SCE], F32, tag="ctx")
                    for kb in range(KB):
                        sc_ps = ps_sc.tile([P, SCE], F32, tag="sc")
                        for j in range(NSUB):
                            nc.tensor.matmul(
                                sc_ps[:, j * SC : (j + 1) * SC],
                                lhsT=kT_sb[ro : ro + DH, et, kb * P : (kb + 1) * P],
                                rhs=qT_sb[
                                    ro : ro + DH,
                                    et,
                                    ch * SCE + j * SC : ch * SCE + (j + 1) * SC,
                                ],
                                start=True,
                                stop=True,
                            )
                        ex = expp.tile([P, SCE], BF16, tag="ex")
                        nc.scalar.activation(
                            ex,
                            sc_ps,
                            mybir.ActivationFunctionType.Exp,
                            scale=sel_sb[:, kb : kb + 1],
                        )
                        for j in range(NSUB):
                            nc.tensor.matmul(
                                ctx_ps[:, j * SC : (j + 1) * SC],
                                lhsT=v_sb[:, kb, h * VW : (h + 1) * VW],
                                rhs=ex[:, j * SC : (j + 1) * SC],
                                start=(kb == 0),
                                stop=(kb == KB - 1),
                            )
                    rec = recp.tile([1, SCE], F32, tag="rec")
                    nc.vector.reciprocal(rec, ctx_ps[DH : DH + 1, :])
                    bc_sb = bcp.tile([DH, SCE], F32, tag="bc")
                    rec_bcast = bass.AP(
                        tensor=rec.tensor,
                        offset=rec.offset,
                        ap=[list(rec.ap[0]), [0, DH], list(rec.ap[1])],
                    )
                    nc.gpsimd.dma_start(bc_sb, rec_bcast)
                    if h % 2 == 0:
                        ct = ctxp.tile([P, SCE], BF16, tag="ct")
                        ctx_tiles.append(ct)
                    nc.vector.tensor_mul(
                        ctx_tiles[et][ro : ro + DH, :], ctx_ps[0:DH, :], bc_sb
                    )
                for ob in range(OB):
                    for j in range(NSUB):
                        ps = ps_sc.tile([P, SC], F32, tag="sc")
                        for et in range(ET):
                            nc.tensor.matmul(
                                ps,
                                lhsT=wo_sb[:, et, ob * P : (ob + 1) * P],
                                rhs=ctx_tiles[et][:, j * SC : (j + 1) * SC],
                                start=(et == 0),
                                stop=(et == ET - 1),
                            )
                        ost = outp.tile([P, SC], F32, tag="ost")
                        nc.vector.tensor_copy(ost, ps)
                        nc.gpsimd.dma_start(
                            outT_d[
                                ob * P : (ob + 1) * P,
                                ch * SCE + j * SC : ch * SCE + (j + 1) * SC,
                            ],
                            ost,
                        )

    _split_multi_waits(nc)
    return nc


def make_in_maps(Q, K, V, valid_lens, Wq, Wk, Wv, Wo):
    """Host-side sharding. Returns 8 in_maps (core c -> batch c//2, group c%2)."""
    xT = {}
    sel = {}
    for b in range(B):
        xT[b] = (
            np.ascontiguousarray(Q[b].T).astype(npbf16),
            np.ascontiguousarray(K[b].T).astype(npbf16),
            np.ascontiguousarray(V[b].T).astype(npbf16),
        )
        s = (np.arange(S) < valid_lens[b]).astype(np.float32)
        sel[b] = np.ascontiguousarray(s.reshape(KB, P).T)
    wshard = {}
    for g in range(2):
        cols = slice(g * E, (g + 1) * E)
        wshard[g] = (
            (Wq[:, cols] / 8.0).astype(npbf16),
            Wk[:, cols].astype(npbf16),
            Wv[:, cols].astype(npbf16),
            np.ascontiguousarray(Wo[cols, :]).astype(npbf16),
        )
    in_maps = []
    for c in range(8):
        b, g = c // 2, c % 2
        wq, wk, wv, wo = wshard[g]
        in_maps.append(
            {
                "xqT": xT[b][0],
                "xkT": xT[b][1],
                "xvT": xT[b][2],
                "wq": wq,
                "wk": wk,
                "wv": wv,
                "wo": wo,
                "sel": sel[b],
            }
        )
    return in_maps


_NC_CACHE = []


def kernel(Q, K, V, valid_lens, Wq, Wk, Wv, Wo):
    Q = np.asarray(Q, dtype=np.float32)
    K = np.asarray(K, dtype=np.float32)
    V = np.asarray(V, dtype=np.float32)
    Wq = np.asarray(Wq, dtype=np.float32)
    Wk = np.asarray(Wk, dtype=np.float32)
    Wv = np.asarray(Wv, dtype=np.float32)
    Wo = np.asarray(Wo, dtype=np.float32)
    valid_lens = np.asarray(valid_lens)

    in_maps = make_in_maps(Q, K, V, valid_lens, Wq, Wk, Wv, Wo)
    if not _NC_CACHE:
        _NC_CACHE.append(build_nc())
    nc = _NC_CACHE[0]
    res = run_bass_kernel_spmd(nc, in_maps, core_ids=list(range(8)))
    outs = [r["outT"] for r in res.results]
    out = np.empty((B, S, D), np.float32)
    for b in range(B):
        out[b] = (outs[2 * b] + outs[2 * b + 1]).T
    return out
